# revision 1
# baseline (speedup 1.0000x reference)
import sys, os
sys.path.insert(0, '/opt/trn_rl_repo')
os.environ.setdefault('JAX_PLATFORMS', 'axon,cpu')
import numpy as np
import ml_dtypes
from contextlib import ExitStack

from concourse import bass, bacc, tile, mybir
from concourse.bass_utils import run_bass_kernel_spmd

BF16 = mybir.dt.bfloat16
F32 = mybir.dt.float32
AF = mybir.ActivationFunctionType

T, B, I, H = 512, 64, 1024, 1024
NC = 8
BL = B // NC            # 8 batch per core
G = 3 * H               # 3072
TB = T * BL             # 4096
KT = H // 128           # 8 k-tiles
MT = G // 128           # 24 m-tiles
UNROLL = 8
NITER = T // UNROLL     # 64

_cache = {}


def _build():
    nc = bacc.Bacc("TRN2", target_bir_lowering=False, debug=False, num_devices=NC)
    dram = {}
    def din(name, shape, dt):
        dram[name] = nc.dram_tensor(name, shape, dt, kind="ExternalInput").ap()
    def dout(name, shape, dt):
        dram[name] = nc.dram_tensor(name, shape, dt, kind="ExternalOutput").ap()
    def dint(name, shape, dt):
        dram[name] = nc.dram_tensor(name, shape, dt).ap()

    din("xT", [H, TB], BF16)
    for l in range(2):
        din(f"wih{l}", [H, G], BF16)   # w_ih_l.T  (lhsT layout [K, M])
        din(f"whh{l}", [H, G], BF16)   # w_hh_l.T
        din(f"bias{l}", [128, MT], F32)  # col mi: b_ih (+ b_hh for r,z)
        din(f"bhhn{l}", [128, BL * KT], F32)  # expanded b_hh n-part
    din("hxT", [2, 128, BL * KT], F32)
    dout("y1T", [H, TB], F32)
    dout("hnT", [2, 128, BL * KT], F32)
    dint("gi0", [128, T * G * BL // 128], BF16)   # [128, 98304] (t-major blocks of 192)
    dint("gi1", [128, T * G * BL // 128], BF16)
    dint("y0T", [H, TB], BF16)

    GW = G * BL // 128  # 192 = step block width in gi layout

    with tile.TileContext(nc) as tc:
        def proj(src_name, w_name, bias_name, gi_name):
            src, w, bias, gi = dram[src_name], dram[w_name], dram[bias_name], dram[gi_name]
            with ExitStack() as ctx:
                wp = ctx.enter_context(tc.tile_pool(name="wp", bufs=1))
                cp = ctx.enter_context(tc.tile_pool(name="cp", bufs=1))
                mp = ctx.enter_context(tc.tile_pool(name="mp", bufs=2))
                sp = ctx.enter_context(tc.tile_pool(name="sp", bufs=2))
                pp = ctx.enter_context(tc.tile_pool(name="pp", bufs=4, space="PSUM"))
                w_sb = []
                for ki in range(KT):
                    wt = wp.tile([128, G], BF16, tag=f"w{ki}")
                    nc.sync.dma_start(wt[:], w[ki * 128:(ki + 1) * 128, :])
                    w_sb.append(wt)
                bias_sb = cp.tile([128, MT], F32, tag="bias")
                nc.sync.dma_start(bias_sb[:], bias[:, :])
                for ch in range(TB // 512):
                    mv = []
                    for ki in range(KT):
                        m = mp.tile([128, 512], BF16, tag=f"m{ki}")
                        nc.sync.dma_start(m[:], src[ki * 128:(ki + 1) * 128, ch * 512:(ch + 1) * 512])
                        mv.append(m)
                    stage = sp.tile([128, 64 * GW], BF16, tag="stage")
                    stage3 = stage.rearrange("p (t g) -> p t g", g=GW)
                    for mi in range(MT):
                        ps = pp.tile([128, 512], F32, tag="ps")
                        for ki in range(KT):
                            nc.tensor.matmul(ps[:], w_sb[ki][:, mi * 128:(mi + 1) * 128], mv[ki][:],
                                             start=(ki == 0), stop=(ki == KT - 1))
                        nc.scalar.activation(stage3[:, :, mi * BL:(mi + 1) * BL],
                                             ps.rearrange("p (t b) -> p t b", b=BL),
                                             AF.Identity, bias=bias_sb[:, mi:mi + 1])
                    nc.sync.dma_start(gi[:, ch * 64 * GW:(ch + 1) * 64 * GW], stage[:])

        def recur(gi_name, w_name, bhhn_name, layer, y_name, y_dt):
            gi, w, bhhn, y = dram[gi_name], dram[w_name], dram[bhhn_name], dram[y_name]
            with ExitStack() as ctx:
                wp = ctx.enter_context(tc.tile_pool(name="rwp", bufs=1))
                cp = ctx.enter_context(tc.tile_pool(name="rcp", bufs=1))
                gp = ctx.enter_context(tc.tile_pool(name="rgp", bufs=4))
                tp = ctx.enter_context(tc.tile_pool(name="rtp", bufs=3))
                yp = ctx.enter_context(tc.tile_pool(name="ryp", bufs=2))
                pp = ctx.enter_context(tc.tile_pool(name="rpp", bufs=3, space="PSUM"))
                w_sb = []
                for ki in range(KT):
                    wt = wp.tile([128, G], BF16, tag=f"rw{ki}")
                    nc.sync.dma_start(wt[:], w[ki * 128:(ki + 1) * 128, :])
                    w_sb.append(wt)
                bhhn_sb = cp.tile([128, BL * KT], F32, tag="bhhn")
                nc.sync.dma_start(bhhn_sb[:], bhhn[:, :])
                h_state = cp.tile([128, BL * KT], F32, tag="hst")
                nc.sync.dma_start(h_state[:], dram["hxT"][layer])
                h_bf = cp.tile([128, BL * KT], BF16, tag="hbf")
                nc.vector.tensor_copy(h_bf[:], h_state[:])

                with tc.For_i(0, NITER, 1, hint_engines=(mybir.EngineType.PE,)) as it:
                    ybuf = yp.tile([128, UNROLL * BL * KT], y_dt, tag="ybuf")
                    for u in range(UNROLL):
                        gi_t = gp.tile([128, GW], BF16, tag="git")
                        nc.sync.dma_start(gi_t[:], gi[:, bass.ds(it * (UNROLL * GW) + u * GW, GW)])
                        ps = pp.tile([128, GW], F32, tag="rps")
                        for mi in range(MT):
                            for ki in range(KT):
                                nc.tensor.matmul(ps[:, mi * BL:(mi + 1) * BL],
                                                 w_sb[ki][:, mi * 128:(mi + 1) * 128],
                                                 h_bf[:, ki * BL:(ki + 1) * BL],
                                                 start=(ki == 0), stop=(ki == KT - 1))
                        RZ = 2 * H * BL // 128  # 128 cols for r+z
                        NW = H * BL // 128      # 64 cols
                        rzs = tp.tile([128, RZ], F32, tag="rzs")
                        nc.vector.tensor_add(rzs[:], ps[:, 0:RZ], gi_t[:, 0:RZ])
                        rz = tp.tile([128, RZ], F32, tag="rz")
                        nc.scalar.activation(rz[:], rzs[:], AF.Sigmoid)
                        hnb = tp.tile([128, NW], F32, tag="hnb")
                        nc.vector.tensor_add(hnb[:], ps[:, RZ:RZ + NW], bhhn_sb[:])
                        rhn = tp.tile([128, NW], F32, tag="rhn")
                        nc.vector.tensor_mul(rhn[:], rz[:, 0:NW], hnb[:])
                        nsum = tp.tile([128, NW], F32, tag="nsum")
                        nc.vector.tensor_add(nsum[:], rhn[:], gi_t[:, RZ:RZ + NW])
                        n_t = tp.tile([128, NW], F32, tag="nt")
                        nc.scalar.activation(n_t[:], nsum[:], AF.Tanh)
                        hmn = tp.tile([128, NW], F32, tag="hmn")
                        nc.vector.tensor_sub(hmn[:], h_state[:], n_t[:])
                        zh = tp.tile([128, NW], F32, tag="zh")
                        nc.vector.tensor_mul(zh[:], rz[:, NW:RZ], hmn[:])
                        nc.vector.tensor_add(h_state[:], n_t[:], zh[:])
                        nc.vector.tensor_copy(h_bf[:], h_state[:])
                        nc.scalar.activation(ybuf[:, u * NW:(u + 1) * NW], h_state[:], AF.Copy)
                    yb4 = ybuf.rearrange("p (t j b) -> p t j b", j=KT, b=BL)
                    for j in range(KT):
                        dst = y[j * 128:(j + 1) * 128, bass.ds(it * (UNROLL * BL), UNROLL * BL)]
                        nc.sync.dma_start(dst.rearrange("p (t b) -> p t b", b=BL), yb4[:, :, j, :])
                nc.sync.dma_start(dram["hnT"][layer], h_state[:])

        proj("xT", "wih0", "bias0", "gi0")
        recur("gi0", "whh0", "bhhn0", 0, "y0T", BF16)
        proj("y0T", "wih1", "bias1", "gi1")
        recur("gi1", "whh1", "bhhn1", 1, "y1T", F32)

    nc.compile()
    return nc


def kernel(x, hx, w_ih_0, w_hh_0, b_ih_0, b_hh_0, w_ih_1, w_hh_1, b_ih_1, b_hh_1, _trace=False):
    if "nc" not in _cache:
        _cache["nc"] = _build()
    nc = _cache["nc"]

    bf = ml_dtypes.bfloat16

    def prep_common():
        m = {}
        for l, (wi, wh, bi, bh) in enumerate([(w_ih_0, w_hh_0, b_ih_0, b_hh_0),
                                              (w_ih_1, w_hh_1, b_ih_1, b_hh_1)]):
            m[f"wih{l}"] = np.ascontiguousarray(np.asarray(wi, np.float32).T).astype(bf)
            m[f"whh{l}"] = np.ascontiguousarray(np.asarray(wh, np.float32).T).astype(bf)
            bias = np.asarray(bi, np.float32).copy()
            bias[:2 * H] += np.asarray(bh, np.float32)[:2 * H]
            m[f"bias{l}"] = np.ascontiguousarray(bias.reshape(MT, 128).T)
            bhn = np.asarray(bh, np.float32)[2 * H:]  # [H]
            m[f"bhhn{l}"] = np.ascontiguousarray(
                np.broadcast_to(bhn.reshape(KT, 128).transpose(1, 0)[:, :, None], (128, KT, BL)).reshape(128, KT * BL))
        return m

    common = prep_common()
    in_maps = []
    for c in range(NC):
        m = dict(common)
        xs = np.asarray(x[:, c * BL:(c + 1) * BL, :], np.float32)  # [T, BL, I]
        m["xT"] = np.ascontiguousarray(xs.transpose(2, 0, 1).reshape(I, TB)).astype(bf)
        hxs = np.asarray(hx[:, c * BL:(c + 1) * BL, :], np.float32)  # [2, BL, H]
        # (l, p, j*BL+b) = hx[l, b, j*128+p]
        m["hxT"] = np.ascontiguousarray(hxs.reshape(2, BL, KT, 128).transpose(0, 3, 2, 1).reshape(2, 128, KT * BL))
        in_maps.append(m)

    res = run_bass_kernel_spmd(nc, in_maps, list(range(NC)), trace=_trace)
    _cache["last_exec_ns"] = res.exec_time_ns

    y = np.empty((T, B, H), np.float32)
    hn = np.empty((2, B, H), np.float32)
    for c in range(NC):
        r = res.results[c]
        # y1T [H, T*BL] -> y[t, c*BL+b, h]
        y[:, c * BL:(c + 1) * BL, :] = r["y1T"].reshape(H, T, BL).transpose(1, 2, 0)
        # hnT [2, 128, KT*BL] -> hn[l, b, j*128+p]
        hn[:, c * BL:(c + 1) * BL, :] = r["hnT"].reshape(2, 128, KT, BL).transpose(0, 3, 2, 1).reshape(2, BL, H)
    return y, hn


# revision 4
# speedup vs baseline: 1.0670x; 1.0670x over previous
import sys, os
sys.path.insert(0, '/opt/trn_rl_repo')
os.environ.setdefault('JAX_PLATFORMS', 'axon,cpu')
import numpy as np
import ml_dtypes
from contextlib import ExitStack

from concourse import bass, bacc, tile, mybir
from concourse.bass_utils import run_bass_kernel_spmd

BF16 = mybir.dt.bfloat16
F32 = mybir.dt.float32
AF = mybir.ActivationFunctionType

T, B, I, H = 512, 64, 1024, 1024
NC = 8
BL = B // NC            # 8 batch per core
G = 3 * H               # 3072
TB = T * BL             # 4096
KT = H // 128           # 8 k-tiles
MT = G // 128           # 24 m-tiles
UNROLL = 8
NITER = T // UNROLL     # 64

_cache = {}


def _build():
    nc = bacc.Bacc("TRN2", target_bir_lowering=False, debug=False, num_devices=NC)
    dram = {}
    def din(name, shape, dt):
        dram[name] = nc.dram_tensor(name, shape, dt, kind="ExternalInput").ap()
    def dout(name, shape, dt):
        dram[name] = nc.dram_tensor(name, shape, dt, kind="ExternalOutput").ap()
    def dint(name, shape, dt):
        dram[name] = nc.dram_tensor(name, shape, dt).ap()

    din("xT", [H, TB], BF16)
    for l in range(2):
        din(f"wih{l}", [H, G], BF16)   # w_ih_l.T  (lhsT layout [K, M])
        din(f"whh{l}", [H, G], BF16)   # w_hh_l.T
        din(f"bias{l}", [128, MT], F32)  # col mi: b_ih (+ b_hh for r,z)
        din(f"bhhn{l}", [128, BL * KT], F32)  # expanded b_hh n-part
    din("hxT", [2, 128, BL * KT], F32)
    dout("y1T", [H, TB], F32)
    dout("hnT", [2, 128, BL * KT], F32)
    dint("gi0", [128, T * G * BL // 128], BF16)   # [128, 98304] (t-major blocks of 192)
    dint("gi1", [128, T * G * BL // 128], BF16)
    dint("y0T", [H, TB], BF16)

    GW = G * BL // 128  # 192 = step block width in gi layout

    with tile.TileContext(nc) as tc:
        def proj(src_name, w_name, bias_name, gi_name):
            src, w, bias, gi = dram[src_name], dram[w_name], dram[bias_name], dram[gi_name]
            with ExitStack() as ctx:
                wp = ctx.enter_context(tc.tile_pool(name="wp", bufs=1))
                cp = ctx.enter_context(tc.tile_pool(name="cp", bufs=1))
                mp = ctx.enter_context(tc.tile_pool(name="mp", bufs=2))
                sp = ctx.enter_context(tc.tile_pool(name="sp", bufs=2))
                pp = ctx.enter_context(tc.tile_pool(name="pp", bufs=4, space="PSUM"))
                w_sb = []
                for ki in range(KT):
                    wt = wp.tile([128, G], BF16, tag=f"w{ki}")
                    nc.sync.dma_start(wt[:], w[ki * 128:(ki + 1) * 128, :])
                    w_sb.append(wt)
                bias_sb = cp.tile([128, MT], F32, tag="bias")
                nc.sync.dma_start(bias_sb[:], bias[:, :])
                for ch in range(TB // 512):
                    mv = []
                    for ki in range(KT):
                        m = mp.tile([128, 512], BF16, tag=f"m{ki}")
                        nc.sync.dma_start(m[:], src[ki * 128:(ki + 1) * 128, ch * 512:(ch + 1) * 512])
                        mv.append(m)
                    stage = sp.tile([128, 64 * GW], BF16, tag="stage")
                    stage3 = stage.rearrange("p (t g) -> p t g", g=GW)
                    for mi in range(MT):
                        ps = pp.tile([128, 512], F32, tag="ps")
                        for ki in range(KT):
                            nc.tensor.matmul(ps[:], w_sb[ki][:, mi * 128:(mi + 1) * 128], mv[ki][:],
                                             start=(ki == 0), stop=(ki == KT - 1))
                        nc.scalar.activation(stage3[:, :, mi * BL:(mi + 1) * BL],
                                             ps.rearrange("p (t b) -> p t b", b=BL),
                                             AF.Identity, bias=bias_sb[:, mi:mi + 1])
                    nc.sync.dma_start(gi[:, ch * 64 * GW:(ch + 1) * 64 * GW], stage[:])

        def recur(gi_name, w_name, bhhn_name, layer, y_name, y_dt):
            gi, w, bhhn, y = dram[gi_name], dram[w_name], dram[bhhn_name], dram[y_name]
            with ExitStack() as ctx:
                wp = ctx.enter_context(tc.tile_pool(name="rwp", bufs=1))
                cp = ctx.enter_context(tc.tile_pool(name="rcp", bufs=1))
                gp = ctx.enter_context(tc.tile_pool(name="rgp", bufs=6))
                tp = ctx.enter_context(tc.tile_pool(name="rtp", bufs=3))
                yp = ctx.enter_context(tc.tile_pool(name="ryp", bufs=2))
                pp = ctx.enter_context(tc.tile_pool(name="rpp", bufs=4, space="PSUM"))
                w_sb = []
                for ki in range(KT):
                    wt = wp.tile([128, G], BF16, tag=f"rw{ki}")
                    nc.sync.dma_start(wt[:], w[ki * 128:(ki + 1) * 128, :])
                    w_sb.append(wt)
                bhhn_sb = cp.tile([128, BL * KT], F32, tag="bhhn")
                nc.sync.dma_start(bhhn_sb[:], bhhn[:, :])
                h_state = cp.tile([128, BL * KT], F32, tag="hst")
                nc.sync.dma_start(h_state[:], dram["hxT"][layer])
                h_bf = cp.tile([128, BL * KT], BF16, tag="hbf")
                nc.vector.tensor_copy(h_bf[:], h_state[:])

                with tc.For_i(0, NITER, 1, hint_engines=(mybir.EngineType.PE,),
                              staggered_reset=True) as it:
                    ybuf = yp.tile([128, UNROLL * BL * KT], y_dt, tag="ybuf")
                    for u in range(UNROLL):
                        gi_t = gp.tile([128, GW], BF16, tag="git")
                        nc.sync.dma_start(gi_t[:], gi[:, bass.ds(it * (UNROLL * GW) + u * GW, GW)])
                        ps = pp.tile([128, GW], F32, tag="rps")
                        for mi in range(MT):
                            for ki in range(KT):
                                nc.tensor.matmul(ps[:, mi * BL:(mi + 1) * BL],
                                                 w_sb[ki][:, mi * 128:(mi + 1) * 128],
                                                 h_bf[:, ki * BL:(ki + 1) * BL],
                                                 start=(ki == 0), stop=(ki == KT - 1))
                        RZ = 2 * H * BL // 128  # 128 cols for r+z
                        NW = H * BL // 128      # 64 cols
                        rzs = tp.tile([128, RZ], F32, tag="rzs")
                        nc.vector.tensor_add(rzs[:], ps[:, 0:RZ], gi_t[:, 0:RZ])
                        rz = tp.tile([128, RZ], F32, tag="rz")
                        nc.scalar.activation(rz[:], rzs[:], AF.Sigmoid)
                        hnb = tp.tile([128, NW], F32, tag="hnb")
                        nc.vector.tensor_add(hnb[:], ps[:, RZ:RZ + NW], bhhn_sb[:])
                        rhn = tp.tile([128, NW], F32, tag="rhn")
                        nc.vector.tensor_mul(rhn[:], rz[:, 0:NW], hnb[:])
                        nsum = tp.tile([128, NW], F32, tag="nsum")
                        nc.vector.tensor_add(nsum[:], rhn[:], gi_t[:, RZ:RZ + NW])
                        n_t = tp.tile([128, NW], F32, tag="nt")
                        nc.scalar.activation(n_t[:], nsum[:], AF.Tanh)
                        hmn = tp.tile([128, NW], F32, tag="hmn")
                        nc.vector.tensor_sub(hmn[:], h_state[:], n_t[:])
                        zh = tp.tile([128, NW], F32, tag="zh")
                        nc.vector.tensor_mul(zh[:], rz[:, NW:RZ], hmn[:])
                        nc.vector.tensor_add(h_state[:], n_t[:], zh[:])
                        nc.vector.tensor_copy(h_bf[:], h_state[:])
                        nc.scalar.activation(ybuf[:, u * NW:(u + 1) * NW], h_state[:], AF.Copy)
                    yb4 = ybuf.rearrange("p (t j b) -> p t j b", j=KT, b=BL)
                    for j in range(KT):
                        dst = y[j * 128:(j + 1) * 128, bass.ds(it * (UNROLL * BL), UNROLL * BL)]
                        nc.sync.dma_start(dst.rearrange("p (t b) -> p t b", b=BL), yb4[:, :, j, :])
                nc.sync.dma_start(dram["hnT"][layer], h_state[:])

        proj("xT", "wih0", "bias0", "gi0")
        recur("gi0", "whh0", "bhhn0", 0, "y0T", BF16)
        proj("y0T", "wih1", "bias1", "gi1")
        recur("gi1", "whh1", "bhhn1", 1, "y1T", F32)

    nc.compile()
    return nc


def kernel(x, hx, w_ih_0, w_hh_0, b_ih_0, b_hh_0, w_ih_1, w_hh_1, b_ih_1, b_hh_1, _trace=False):
    if "nc" not in _cache:
        _cache["nc"] = _build()
    nc = _cache["nc"]

    bf = ml_dtypes.bfloat16

    def prep_common():
        m = {}
        for l, (wi, wh, bi, bh) in enumerate([(w_ih_0, w_hh_0, b_ih_0, b_hh_0),
                                              (w_ih_1, w_hh_1, b_ih_1, b_hh_1)]):
            m[f"wih{l}"] = np.ascontiguousarray(np.asarray(wi, np.float32).T).astype(bf)
            m[f"whh{l}"] = np.ascontiguousarray(np.asarray(wh, np.float32).T).astype(bf)
            bias = np.asarray(bi, np.float32).copy()
            bias[:2 * H] += np.asarray(bh, np.float32)[:2 * H]
            m[f"bias{l}"] = np.ascontiguousarray(bias.reshape(MT, 128).T)
            bhn = np.asarray(bh, np.float32)[2 * H:]  # [H]
            m[f"bhhn{l}"] = np.ascontiguousarray(
                np.broadcast_to(bhn.reshape(KT, 128).transpose(1, 0)[:, :, None], (128, KT, BL)).reshape(128, KT * BL))
        return m

    common = prep_common()
    in_maps = []
    for c in range(NC):
        m = dict(common)
        xs = np.asarray(x[:, c * BL:(c + 1) * BL, :], np.float32)  # [T, BL, I]
        m["xT"] = np.ascontiguousarray(xs.transpose(2, 0, 1).reshape(I, TB)).astype(bf)
        hxs = np.asarray(hx[:, c * BL:(c + 1) * BL, :], np.float32)  # [2, BL, H]
        # (l, p, j*BL+b) = hx[l, b, j*128+p]
        m["hxT"] = np.ascontiguousarray(hxs.reshape(2, BL, KT, 128).transpose(0, 3, 2, 1).reshape(2, 128, KT * BL))
        in_maps.append(m)

    res = run_bass_kernel_spmd(nc, in_maps, list(range(NC)), trace=_trace)
    _cache["last_exec_ns"] = res.exec_time_ns

    y = np.empty((T, B, H), np.float32)
    hn = np.empty((2, B, H), np.float32)
    for c in range(NC):
        r = res.results[c]
        # y1T [H, T*BL] -> y[t, c*BL+b, h]
        y[:, c * BL:(c + 1) * BL, :] = r["y1T"].reshape(H, T, BL).transpose(1, 2, 0)
        # hnT [2, 128, KT*BL] -> hn[l, b, j*128+p]
        hn[:, c * BL:(c + 1) * BL, :] = r["hnT"].reshape(2, 128, KT, BL).transpose(0, 3, 2, 1).reshape(2, BL, H)
    return y, hn
